# revision 21
# baseline (speedup 1.0000x reference)
"""Trainium2 Bass kernel for a dense pre-norm transformer block.

Reference semantics (per batch b, seq len T=100, d_model D=384, heads H=6):
  h   = LN(x) * g1 + beta1
  q,k,v = per-head projections of h;  wei = softmax(mask(q k^T * sqrt(64)))
  x2  = x + (wei v) Wp + bp
  out = x2 + relu(LN(x2)*g2+beta2 @ W1 + b1) @ W2 + b2

Distribution: data-parallel over the batch dim across 8 NeuronCores
(64 batches each); weights are replicated. No collectives.

Key design points:
- LN affines are folded into the adjacent matmul weights/biases on the
  host; the sqrt(head_size) score scale is folded into Wq.
- The whole matmul path runs in bf16 (fp32 PSUM accumulation): bf16
  gets full PE rate AND hardware fast-weight-load with overlapped
  LDWEIGHTS (fp32/fp32r matmuls must self-load their stationary
  operand, serializing ~107ns per matmul). Residuals/statistics stay
  fp32.
- Attention avoids per-head transposes: scores are computed transposed
  (K^T-slice stationary x Q^T-slice moving), the softmax denominator
  comes from ones-columns appended to V inside the AV matmul, and the
  1/den scaling happens token-major where it is a per-partition scalar.
  Head triples share a PE row-group so same-bank PSUM writes stay
  sequential (concurrent same-bank writes are a hardware fault).
- Software pipeline: group g's LN1 runs ahead; its Q/K/V projections
  interleave with group g-1's attention so the tensor engine never
  idles long enough for the HAM clock gate to re-throttle. Exp and
  Sqrt live in different ACT table sets, so LN phases are kept
  clustered instead of finely alternating with attention exp calls.
"""

import numpy as np
from contextlib import ExitStack

B, T, D = 512, 100, 384
H, HS = 6, 64
DH = 4 * D                      # FFN hidden 1536
N_CORES = 8
BC = B // N_CORES               # batches per core
EPS = 1e-5
MASK_VAL = -1e9
NB = 5                          # batches per inner group
NTOK_MAX = NB * T               # 500
KC_D = D // 128                 # 3 contraction chunks over D
KC_H = DH // 128                # 12 contraction chunks over DH
VW = 66                         # V columns per head: 64 + 2 ones (denominator)

_NC_CACHE = {}


def _build_nc(use_bv, use_bp, use_b2):
    import concourse.bass as bass
    import concourse.tile as tile
    from concourse import bacc, mybir

    f32 = mybir.dt.float32
    bf16 = mybir.dt.bfloat16
    AF = mybir.ActivationFunctionType
    OP = mybir.AluOpType
    ts = bass.ts

    nc = bacc.Bacc("TRN2", target_bir_lowering=False, debug=False,
                   enable_asserts=True, num_devices=N_CORES)

    x_d = nc.dram_tensor("x", [BC, T, D], f32, kind="ExternalInput").ap()
    wq_d = nc.dram_tensor("wq_l", [128, KC_D, D], bf16, kind="ExternalInput").ap()
    wk_d = nc.dram_tensor("wk_l", [128, KC_D, D], bf16, kind="ExternalInput").ap()
    wv_d = nc.dram_tensor("wv_l", [128, KC_D, D], bf16, kind="ExternalInput").ap()
    wp_d = nc.dram_tensor("wp_l", [128, KC_D, D], bf16, kind="ExternalInput").ap()
    w1_d = nc.dram_tensor("w1_l", [128, KC_D, DH], bf16, kind="ExternalInput").ap()
    w2_d = nc.dram_tensor("w2_l", [128, KC_H, D], bf16, kind="ExternalInput").ap()
    bq_d = nc.dram_tensor("bq_l", [128, KC_D], f32, kind="ExternalInput").ap()
    bk_d = nc.dram_tensor("bk_l", [128, KC_D], f32, kind="ExternalInput").ap()
    b1_d = nc.dram_tensor("b1_l", [128, KC_H], f32, kind="ExternalInput").ap()
    mask_d = nc.dram_tensor("mask3", [T, 3 * T], f32, kind="ExternalInput").ap()
    id_d = nc.dram_tensor("ident", [128, 128], bf16, kind="ExternalInput").ap()
    bv_d = bp_d = b2_d = None
    if use_bv:
        bv_d = nc.dram_tensor("bv_bc", [128, D], f32, kind="ExternalInput").ap()
    if use_bp:
        bp_d = nc.dram_tensor("bp_bc", [128, D], f32, kind="ExternalInput").ap()
    if use_b2:
        b2_d = nc.dram_tensor("b2_bc", [128, D], f32, kind="ExternalInput").ap()
    out_d = nc.dram_tensor("out", [BC, T, D], f32, kind="ExternalOutput").ap()

    with tile.TileContext(nc) as tc, ExitStack() as ctx:
        wpool = ctx.enter_context(tc.tile_pool(name="wpool", bufs=1))
        px = ctx.enter_context(tc.tile_pool(name="px", bufs=12))
        pxn = ctx.enter_context(tc.tile_pool(name="pxn", bufs=2))
        pst = ctx.enter_context(tc.tile_pool(name="pst", bufs=4))
        pxt = ctx.enter_context(tc.tile_pool(name="pxt", bufs=2))
        pqk = ctx.enter_context(tc.tile_pool(name="pqk", bufs=2))
        pv = ctx.enter_context(tc.tile_pool(name="pv", bufs=2))
        patt = ctx.enter_context(tc.tile_pool(name="patt", bufs=3))
        po = ctx.enter_context(tc.tile_pool(name="po", bufs=2))
        pot = ctx.enter_context(tc.tile_pool(name="pot", bufs=1))
        px2 = ctx.enter_context(tc.tile_pool(name="px2", bufs=1))
        phid = ctx.enter_context(tc.tile_pool(name="phid", bufs=1))
        pout = ctx.enter_context(tc.tile_pool(name="pout", bufs=3))
        # PSUM: 8 banks total -> (pool,tag) bufs sum to 8
        psq = ctx.enter_context(tc.tile_pool(name="psq", bufs=2, space="PSUM"))
        pstk = ctx.enter_context(tc.tile_pool(name="pstk", bufs=2, space="PSUM"))
        psa = ctx.enter_context(tc.tile_pool(name="psa", bufs=2, space="PSUM"))
        psv = ctx.enter_context(tc.tile_pool(name="psv", bufs=2, space="PSUM"))

        # resident weights / constants
        wq_sb = wpool.tile([128, KC_D, D], bf16)
        nc.sync.dma_start(out=wq_sb[:], in_=wq_d[:])
        wk_sb = wpool.tile([128, KC_D, D], bf16)
        nc.sync.dma_start(out=wk_sb[:], in_=wk_d[:])
        wv_sb = wpool.tile([128, KC_D, D], bf16)
        nc.sync.dma_start(out=wv_sb[:], in_=wv_d[:])
        wp_sb = wpool.tile([128, KC_D, D], bf16)
        nc.sync.dma_start(out=wp_sb[:], in_=wp_d[:])
        w1_sb = wpool.tile([128, KC_D, DH], bf16)
        nc.sync.dma_start(out=w1_sb[:], in_=w1_d[:])
        w2_sb = wpool.tile([128, KC_H, D], bf16)
        nc.sync.dma_start(out=w2_sb[:], in_=w2_d[:])
        bq_sb = wpool.tile([128, KC_D], f32)
        nc.sync.dma_start(out=bq_sb[:], in_=bq_d[:])
        bk_sb = wpool.tile([128, KC_D], f32)
        nc.sync.dma_start(out=bk_sb[:], in_=bk_d[:])
        b1_sb = wpool.tile([128, KC_H], f32)
        nc.sync.dma_start(out=b1_sb[:], in_=b1_d[:])
        mask_sb = wpool.tile([T, 3 * T], f32)
        nc.sync.dma_start(out=mask_sb[:], in_=mask_d[:])
        id_sb = wpool.tile([128, 128], bf16)
        nc.sync.dma_start(out=id_sb[:], in_=id_d[:])
        eps_sb = wpool.tile([128, 1], f32)
        nc.vector.memset(eps_sb[:], EPS)
        bv_sb = bp_sb = b2_sb = None
        if use_bv:
            bv_sb = wpool.tile([128, D], f32)
            nc.sync.dma_start(out=bv_sb[:], in_=bv_d[:])
        if use_bp:
            bp_sb = wpool.tile([128, D], f32)
            nc.sync.dma_start(out=bp_sb[:], in_=bp_d[:])
        if use_b2:
            b2_sb = wpool.tile([128, D], f32)
            nc.sync.dma_start(out=b2_sb[:], in_=b2_d[:])

        assert D <= nc.vector.BN_STATS_FMAX

        def ln_stats(x_sl, tp, mvall, i):
            st = pst.tile([128, nc.vector.BN_STATS_DIM], f32, tag="bnst")
            nc.vector.bn_stats(out=st[:tp], in_=x_sl)
            nc.vector.bn_aggr(out=mvall[:tp, :, i], in_=st[:tp])

        def ln_finish2(mvall, tp, half, tag):
            # Sqrt + reciprocal for two tiles of the phase
            sl = slice(2 * half, 2 * half + 2)
            rs2 = pst.tile([128, 2], f32, tag=tag + f"rs{half}")
            nc.scalar.activation(out=rs2[:tp], in_=mvall[:tp, 1, sl], func=AF.Sqrt,
                                 bias=eps_sb[:tp], scale=1.0)
            nc.vector.reciprocal(out=rs2[:tp], in_=rs2[:tp])
            nm2 = pst.tile([128, 2], f32, tag=tag + f"nm{half}")
            nc.vector.scalar_tensor_tensor(out=nm2[:tp], in0=mvall[:tp, 0, sl],
                                           scalar=-1.0, in1=rs2[:tp],
                                           op0=OP.mult, op1=OP.mult)
            return rs2, nm2

        def ln_apply(x_sl, tp, rs2, nm2, j, i, dstT, tag):
            xn = pxn.tile([128, D], bf16, tag=tag)
            nc.scalar.activation(out=xn[:tp], in_=x_sl, func=AF.Identity,
                                 bias=nm2[:tp, j:j + 1], scale=rs2[:tp, j:j + 1])
            ps = psq.tile([128, KC_D, 128], bf16, tag="ps_big")
            for c in range(KC_D):
                nc.tensor.transpose(ps[:128, c, :tp],
                                    xn[:tp, ts(c, 128)], id_sb[:tp, :tp])
            nc.vector.tensor_copy(dstT[:, :, i * tp:(i + 1) * tp], ps[:, :, :tp])

        def preload_act_table(func):
            # 1-element dummy activation: hoists the ACT table-set DMA off
            # the critical path (walrus loads a set right before first use)
            dj = pst.tile([128, 1], f32, tag="dummy")
            nc.scalar.activation(out=dj[:1], in_=eps_sb[:1], func=func)

        xf = x_d.flatten_outer_dims()
        of = out_d.flatten_outer_dims()

        groups = []
        tok = 0
        nbat = BC
        while nbat > 0:
            nb = min(NB, nbat)
            groups.append((tok, nb))
            tok += nb * T
            nbat -= nb

        live = {}

        def emit_ln1(gi):
            tok0, nb = groups[gi]
            ntok = nb * T
            tp = ntok // 4
            XnT = pxt.tile([128, KC_D, NTOK_MAX], bf16, tag="xnt")
            mvall = pst.tile([128, nc.vector.BN_AGGR_DIM, 4], f32, tag="xn1mv")
            xres = []
            for i in range(4):
                xt_ = px.tile([128, D], f32, tag="xres")
                nc.sync.dma_start(out=xt_[:tp],
                                  in_=xf[tok0 + i * tp: tok0 + (i + 1) * tp])
                xres.append(xt_)
            for half in range(2):
                for i in (2 * half, 2 * half + 1):
                    ln_stats(xres[i][:tp], tp, mvall, i)
                    yield
                rs2, nm2 = ln_finish2(mvall, tp, half, "xn1")
                for j, i in enumerate((2 * half, 2 * half + 1)):
                    ln_apply(xres[i][:tp], tp, rs2, nm2, j, i, XnT, "xn1")
                    yield
            live[gi] = dict(xres=xres, XnT=XnT)
            yield

        def emit_qkv(gi):
            """Q/K/V projections for group gi. Generator (PE-dense filler)."""
            tok0, nb = groups[gi]
            ntok = nb * T
            XnT = live[gi]["XnT"]
            QT = pqk.tile([128, KC_D, NTOK_MAX], bf16, tag="qt")
            KT = pqk.tile([128, KC_D, NTOK_MAX], bf16, tag="kt")
            for di, (dst, w_sb, b_sb) in enumerate(((QT, wq_sb, bq_sb),
                                                    (KT, wk_sb, bk_sb))):
                for m in range(KC_D):
                    ps = psq.tile([128, NTOK_MAX], f32, tag="ps_big")
                    for kc in range(KC_D):
                        nc.tensor.matmul(ps[:, :ntok], w_sb[:, kc, ts(m, 128)],
                                         XnT[:, kc, :ntok],
                                         start=(kc == 0), stop=(kc == KC_D - 1))
                    if di == 0:
                        nc.scalar.activation(out=dst[:, m, :ntok], in_=ps[:, :ntok],
                                             func=AF.Identity, bias=b_sb[:, m:m + 1],
                                             scale=1.0)
                    else:
                        nc.vector.tensor_scalar_add(out=dst[:, m, :ntok],
                                                    in0=ps[:, :ntok],
                                                    scalar1=b_sb[:, m:m + 1])
                yield
            V = pv.tile([128, NB, H, VW], bf16, tag="v")
            nc.vector.memset(V[:T, :nb, :, 64:VW], 1.0)
            for b in range(nb):
                ps = pstk.tile([128, D], f32, tag="ps_tok")
                for kc in range(KC_D):
                    nc.tensor.matmul(ps[:T, :], XnT[:, kc, b * T:(b + 1) * T],
                                     wv_sb[:, kc, :],
                                     start=(kc == 0), stop=(kc == KC_D - 1))
                psh = ps[:T].rearrange("p (h s) -> p h s", h=H)
                if use_bv:
                    bvh = bv_sb[:T].rearrange("p (h s) -> p h s", h=H)
                    nc.vector.tensor_add(V[:T, b, :, 0:64], psh, bvh)
                else:
                    nc.vector.tensor_copy(V[:T, b, :, 0:64], psh)
                if b % 2 == 1:
                    yield
            live[gi].update(QT=QT, KT=KT, V=V)
            yield

        def emit_attn(gi):
            """Attention for group gi -> feature-major OT. Yields per batch."""
            tok0, nb = groups[gi]
            QT, KT, V = live[gi]["QT"], live[gi]["KT"], live[gi]["V"]
            OT = pot.tile([128, KC_D, NTOK_MAX], bf16, tag="ot")
            for b in range(nb):
                bs = slice(b * T, (b + 1) * T)
                O_sb = po.tile([128, D], bf16, tag="o")
                rden = pst.tile([128, H], f32, tag="rden")
                for g3 in range(2):
                    # same (h%2) within a triple: one PE row-group, so the
                    # three same-bank matmuls issue sequentially (concurrent
                    # same-bank PSUM writes are a hardware fault)
                    hs3 = [g3, g3 + 2, g3 + 4]
                    ps_s = psa.tile([128, 3, T], f32, tag="ps_att")
                    for j, h in enumerate(hs3):
                        mb, mo = h // 2, (h % 2) * 64
                        nc.tensor.matmul(ps_s[:T, j, :],
                                         KT[mo:mo + 64, mb, bs],
                                         QT[mo:mo + 64, mb, bs],
                                         start=True, stop=True)
                    sm = patt.tile([128, 3 * T], f32, tag="sm")
                    nc.vector.tensor_add(sm[:T], ps_s[:T, :, :], mask_sb[:T])
                    ex = patt.tile([128, 3 * T], bf16, tag="ex")
                    nc.scalar.activation(out=ex[:T], in_=sm[:T], func=AF.Exp)
                    ps_o = psv.tile([128, 3, VW], f32, tag="ps_av")
                    for j, h in enumerate(hs3):
                        nc.tensor.matmul(ps_o[:T, j, :],
                                         ex[:T, ts(j, T)],
                                         V[:T, b, h, :],
                                         start=True, stop=True)
                    nc.vector.reciprocal(out=rden[:T, g3 * 3:(g3 + 1) * 3],
                                         in_=ps_o[:T, :, 64:65])
                    # one mul for the whole triple: rden broadcast along the
                    # 64-wide head slice via a stride-0 inner AP dim
                    rsl = rden[:T, g3 * 3:(g3 + 1) * 3]
                    rb = bass.AP(tensor=rsl.tensor, offset=rsl.offset,
                                 ap=[list(rsl.ap[0]), list(rsl.ap[1]), [0, 64]])
                    osl = O_sb[:T].rearrange("p (c two s) -> p c two s",
                                             two=2, s=64)[:, :, g3, :]
                    nc.vector.tensor_mul(osl, ps_o[:T, :, 0:64], rb)
                ps_t = psa.tile([128, KC_D, T], bf16, tag="ps_att")
                for c in range(KC_D):
                    nc.tensor.transpose(ps_t[:128, c, :],
                                        O_sb[:T, ts(c, 128)], id_sb[:T, :T])
                nc.scalar.copy(out=OT[:, :, bs], in_=ps_t[:, :, :])
                yield
            live[gi]["OT"] = OT

        def emit_tail(gi):
            """proj+residual, LN2, FFN, store for group gi. Generator."""
            tok0, nb = groups[gi]
            ntok = nb * T
            tp = ntok // 4
            xres, OT = live[gi]["xres"], live[gi]["OT"]
            preload_act_table(AF.Sqrt)
            X2 = px2.tile([128, 4, D], f32, tag="x2")
            mvall = pst.tile([128, nc.vector.BN_AGGR_DIM, 4], f32, tag="xn2mv")
            Xn2T = pxt.tile([128, KC_D, NTOK_MAX], bf16, tag="xn2t")
            rsnm = {}
            for i in range(4):
                ps = pstk.tile([128, D], f32, tag="ps_tok")
                for kc in range(KC_D):
                    nc.tensor.matmul(ps[:tp, :], OT[:, kc, i * tp:(i + 1) * tp],
                                     wp_sb[:, kc, :],
                                     start=(kc == 0), stop=(kc == KC_D - 1))
                if use_bp:
                    nc.vector.tensor_add(ps[:tp, :], ps[:tp, :], bp_sb[:tp, :])
                nc.vector.tensor_add(X2[:tp, i, :], ps[:tp, :], xres[i][:tp, :])
                ln_stats(X2[:tp, i, :], tp, mvall, i)
                if i == 1:
                    rsnm[0] = ln_finish2(mvall, tp, 0, "xn2")
                    ln_apply(X2[:tp, 0, :], tp, *rsnm[0], 0, 0, Xn2T, "xn2")
                if i == 3:
                    ln_apply(X2[:tp, 1, :], tp, *rsnm[0], 1, 1, Xn2T, "xn2")
                    rsnm[1] = ln_finish2(mvall, tp, 1, "xn2")
                    ln_apply(X2[:tp, 2, :], tp, *rsnm[1], 0, 2, Xn2T, "xn2")
                    ln_apply(X2[:tp, 3, :], tp, *rsnm[1], 1, 3, Xn2T, "xn2")
                yield
            preload_act_table(AF.Exp)
            hf = 2 * tp
            HT = phid.tile([128, KC_H, NTOK_MAX], bf16, tag="hid")
            for m in range(KC_H):
                ps = psq.tile([128, NTOK_MAX], f32, tag="ps_big")
                for kc in range(KC_D):
                    nc.tensor.matmul(ps[:, :hf], w1_sb[:, kc, ts(m, 128)],
                                     Xn2T[:, kc, :hf],
                                     start=(kc == 0), stop=(kc == KC_D - 1))
                for kc in range(KC_D):
                    nc.tensor.matmul(ps[:, hf:ntok], w1_sb[:, kc, ts(m, 128)],
                                     Xn2T[:, kc, hf:ntok],
                                     start=(kc == 0), stop=(kc == KC_D - 1))
                if m % 2 == 0:
                    nc.scalar.activation(out=HT[:, m, :ntok], in_=ps[:, :ntok],
                                         func=AF.Relu, bias=b1_sb[:, m:m + 1],
                                         scale=1.0)
                else:
                    nc.vector.tensor_scalar(out=HT[:, m, :ntok], in0=ps[:, :ntok],
                                            scalar1=b1_sb[:, m:m + 1], scalar2=0.0,
                                            op0=OP.add, op1=OP.max)
                if m % 2 == 1:
                    yield
            for i in range(4):
                ps = pstk.tile([128, D], f32, tag="ps_tok")
                for kc in range(KC_H):
                    nc.tensor.matmul(ps[:tp, :], HT[:, kc, i * tp:(i + 1) * tp],
                                     w2_sb[:, kc, :],
                                     start=(kc == 0), stop=(kc == KC_H - 1))
                if use_b2:
                    nc.vector.tensor_add(ps[:tp, :], ps[:tp, :], b2_sb[:tp, :])
                ot_ = pout.tile([128, D], f32, tag="outt")
                nc.vector.tensor_add(ot_[:tp, :], ps[:tp, :], X2[:tp, i, :])
                nc.sync.dma_start(out=of[tok0 + i * tp: tok0 + (i + 1) * tp],
                                  in_=ot_[:tp, :])
                yield
            del live[gi]

        def alternate(it_a, it_b):
            while it_a is not None or it_b is not None:
                if it_a is not None:
                    try:
                        next(it_a)
                    except StopIteration:
                        it_a = None
                if it_b is not None:
                    try:
                        next(it_b)
                    except StopIteration:
                        it_b = None

        # software pipeline, per iteration g:
        #   [attention(g-1) x QKV(g)]  then  [tail(g-1) x LN1(g+1)]
        # LN1's PE-light stats run under the FFN-dense tail; attention's
        # gap-prone phase runs under the QKV matmuls.
        for _ in emit_ln1(0):
            pass
        for g in range(len(groups)):
            alternate(emit_attn(g - 1) if g >= 1 else None, emit_qkv(g))
            alternate(emit_tail(g - 1) if g >= 1 else None,
                      emit_ln1(g + 1) if g + 1 < len(groups) else None)
        alternate(emit_attn(len(groups) - 1), None)
        alternate(emit_tail(len(groups) - 1), None)

    nc.compile()
    return nc


def _get_nc(use_bv, use_bp, use_b2):
    key = (use_bv, use_bp, use_b2)
    if key not in _NC_CACHE:
        _NC_CACHE[key] = _build_nc(*key)
    return _NC_CACHE[key]


def _prep_inputs(x, wq, wk, wv, wproj, bproj, w1, b1, w2, b2, g1, beta1, g2, beta2):
    import ml_dtypes
    f = np.float32
    bf = ml_dtypes.bfloat16
    # stack per-head projections into [D, D] with head h at columns h*HS:(h+1)*HS
    wq_f = np.ascontiguousarray(wq.transpose(1, 0, 2).reshape(D, D), dtype=f)
    wk_f = np.ascontiguousarray(wk.transpose(1, 0, 2).reshape(D, D), dtype=f)
    wv_f = np.ascontiguousarray(wv.transpose(1, 0, 2).reshape(D, D), dtype=f)
    scale = np.float32(HS ** 0.5)
    # fold LN1 affine into qkv weights, LN2 affine into w1
    wq_p = (g1[:, None] * wq_f) * scale
    wk_p = g1[:, None] * wk_f
    wv_p = g1[:, None] * wv_f
    w1_p = g2[:, None] * w1
    bq = (beta1 @ wq_f) * scale
    bk = beta1 @ wk_f
    bv = beta1 @ wv_f
    b1_p = b1 + beta2 @ w1
    bp = bproj
    b2_p = b2

    def lay(w, kc):
        # [K, M] -> [128, kc, M] bf16 with K split into kc chunks of 128
        return np.ascontiguousarray(
            np.asarray(w, dtype=f).reshape(kc, 128, w.shape[1]).transpose(1, 0, 2)
        ).astype(bf)

    def layb(bias, kc):
        return np.ascontiguousarray(bias.reshape(kc, 128).T, dtype=f)

    # transposed causal mask, tiled for 3 heads: keep (t >= u)
    maskT = np.full((T, T), MASK_VAL, dtype=f)
    maskT[np.triu_indices(T)] = 0.0
    mask3 = np.ascontiguousarray(np.tile(maskT, (1, 3)))

    shared = {
        "wq_l": lay(wq_p, KC_D), "wk_l": lay(wk_p, KC_D), "wv_l": lay(wv_p, KC_D),
        "wp_l": lay(wproj, KC_D), "w1_l": lay(w1_p, KC_D), "w2_l": lay(w2, KC_H),
        "bq_l": layb(bq, KC_D), "bk_l": layb(bk, KC_D), "b1_l": layb(b1_p, KC_H),
        "mask3": mask3, "ident": np.eye(128, dtype=f).astype(bf),
    }
    use_bv = bool(np.any(bv))
    use_bp = bool(np.any(bp))
    use_b2 = bool(np.any(b2_p))
    if use_bv:
        shared["bv_bc"] = np.ascontiguousarray(np.tile(bv.astype(f), (128, 1)))
    if use_bp:
        shared["bp_bc"] = np.ascontiguousarray(np.tile(np.asarray(bp, f), (128, 1)))
    if use_b2:
        shared["b2_bc"] = np.ascontiguousarray(np.tile(np.asarray(b2_p, f), (128, 1)))
    return shared, (use_bv, use_bp, use_b2)


def kernel(**inputs):
    from concourse.bass_utils import run_bass_kernel_spmd

    x = np.asarray(inputs["x"], dtype=np.float32)
    shared, flags = _prep_inputs(
        x, *[np.asarray(inputs[k], dtype=np.float32) for k in
             ("wq", "wk", "wv", "wproj", "bproj", "w1", "b1", "w2", "b2",
              "g1", "beta1", "g2", "beta2")])
    nc = _get_nc(*flags)
    in_maps = []
    for c in range(N_CORES):
        m = dict(shared)
        m["x"] = np.ascontiguousarray(x[c * BC:(c + 1) * BC])
        in_maps.append(m)
    res = run_bass_kernel_spmd(nc, in_maps, core_ids=list(range(N_CORES)))
    return np.concatenate([res.results[i]["out"] for i in range(N_CORES)], axis=0)


# revision 22
# speedup vs baseline: 1.0237x; 1.0237x over previous
"""Trainium2 Bass kernel for a dense pre-norm transformer block.

Reference semantics (per batch b, seq len T=100, d_model D=384, heads H=6):
  h   = LN(x) * g1 + beta1
  q,k,v = per-head projections of h;  wei = softmax(mask(q k^T * sqrt(64)))
  x2  = x + (wei v) Wp + bp
  out = x2 + relu(LN(x2)*g2+beta2 @ W1 + b1) @ W2 + b2

Distribution: data-parallel over the batch dim across 8 NeuronCores
(64 batches each); weights are replicated. No collectives.

Key design points:
- LN affines are folded into the adjacent matmul weights/biases on the
  host; the sqrt(head_size) score scale is folded into Wq.
- The whole matmul path runs in bf16 (fp32 PSUM accumulation): bf16
  gets full PE rate AND hardware fast-weight-load with overlapped
  LDWEIGHTS (fp32/fp32r matmuls must self-load their stationary
  operand, serializing ~107ns per matmul). Residuals/statistics stay
  fp32.
- Attention avoids per-head transposes: scores are computed transposed
  (K^T-slice stationary x Q^T-slice moving), the softmax denominator
  comes from ones-columns appended to V inside the AV matmul, and the
  1/den scaling happens token-major where it is a per-partition scalar.
  Head triples share a PE row-group so same-bank PSUM writes stay
  sequential (concurrent same-bank writes are a hardware fault).
- Software pipeline: group g's LN1 runs ahead; its Q/K/V projections
  interleave with group g-1's attention so the tensor engine never
  idles long enough for the HAM clock gate to re-throttle. Exp and
  Sqrt live in different ACT table sets, so LN phases are kept
  clustered instead of finely alternating with attention exp calls.
"""

import numpy as np
from contextlib import ExitStack

B, T, D = 512, 100, 384
H, HS = 6, 64
DH = 4 * D                      # FFN hidden 1536
N_CORES = 8
BC = B // N_CORES               # batches per core
EPS = 1e-5
MASK_VAL = -1e9
NB = 5                          # batches per inner group
NTOK_MAX = NB * T               # 500
KC_D = D // 128                 # 3 contraction chunks over D
KC_H = DH // 128                # 12 contraction chunks over DH
VW = 66                         # V columns per head: 64 + 2 ones (denominator)

_NC_CACHE = {}


def _build_nc(use_bv, use_bp, use_b2):
    import concourse.bass as bass
    import concourse.tile as tile
    from concourse import bacc, mybir

    f32 = mybir.dt.float32
    bf16 = mybir.dt.bfloat16
    AF = mybir.ActivationFunctionType
    OP = mybir.AluOpType
    ts = bass.ts

    nc = bacc.Bacc("TRN2", target_bir_lowering=False, debug=False,
                   enable_asserts=True, num_devices=N_CORES)

    x_d = nc.dram_tensor("x", [BC, T, D], f32, kind="ExternalInput").ap()
    wq_d = nc.dram_tensor("wq_l", [128, KC_D, D], bf16, kind="ExternalInput").ap()
    wk_d = nc.dram_tensor("wk_l", [128, KC_D, D], bf16, kind="ExternalInput").ap()
    wv_d = nc.dram_tensor("wv_l", [128, KC_D, D], bf16, kind="ExternalInput").ap()
    wp_d = nc.dram_tensor("wp_l", [128, KC_D, D], bf16, kind="ExternalInput").ap()
    w1_d = nc.dram_tensor("w1_l", [128, KC_D, DH], bf16, kind="ExternalInput").ap()
    w2_d = nc.dram_tensor("w2_l", [128, KC_H, D], bf16, kind="ExternalInput").ap()
    bq_d = nc.dram_tensor("bq_l", [128, KC_D], f32, kind="ExternalInput").ap()
    bk_d = nc.dram_tensor("bk_l", [128, KC_D], f32, kind="ExternalInput").ap()
    b1_d = nc.dram_tensor("b1_l", [128, KC_H], f32, kind="ExternalInput").ap()
    mask_d = nc.dram_tensor("mask3", [T, 3 * T], f32, kind="ExternalInput").ap()
    id_d = nc.dram_tensor("ident", [128, 128], bf16, kind="ExternalInput").ap()
    bv_d = bp_d = b2_d = None
    if use_bv:
        bv_d = nc.dram_tensor("bv_bc", [128, D], f32, kind="ExternalInput").ap()
    if use_bp:
        bp_d = nc.dram_tensor("bp_bc", [128, D], f32, kind="ExternalInput").ap()
    if use_b2:
        b2_d = nc.dram_tensor("b2_bc", [128, D], f32, kind="ExternalInput").ap()
    out_d = nc.dram_tensor("out", [BC, T, D], f32, kind="ExternalOutput").ap()

    with tile.TileContext(nc) as tc, ExitStack() as ctx:
        wpool = ctx.enter_context(tc.tile_pool(name="wpool", bufs=1))
        px = ctx.enter_context(tc.tile_pool(name="px", bufs=12))
        pxn = ctx.enter_context(tc.tile_pool(name="pxn", bufs=2))
        pst = ctx.enter_context(tc.tile_pool(name="pst", bufs=4))
        pxt = ctx.enter_context(tc.tile_pool(name="pxt", bufs=2))
        pqk = ctx.enter_context(tc.tile_pool(name="pqk", bufs=2))
        pv = ctx.enter_context(tc.tile_pool(name="pv", bufs=2))
        patt = ctx.enter_context(tc.tile_pool(name="patt", bufs=3))
        po = ctx.enter_context(tc.tile_pool(name="po", bufs=2))
        pot = ctx.enter_context(tc.tile_pool(name="pot", bufs=1))
        px2 = ctx.enter_context(tc.tile_pool(name="px2", bufs=1))
        phid = ctx.enter_context(tc.tile_pool(name="phid", bufs=1))
        pout = ctx.enter_context(tc.tile_pool(name="pout", bufs=3))
        # PSUM: 8 banks total -> (pool,tag) bufs sum to 8
        psq = ctx.enter_context(tc.tile_pool(name="psq", bufs=2, space="PSUM"))
        pstk = ctx.enter_context(tc.tile_pool(name="pstk", bufs=2, space="PSUM"))
        psa = ctx.enter_context(tc.tile_pool(name="psa", bufs=2, space="PSUM"))
        psv = ctx.enter_context(tc.tile_pool(name="psv", bufs=2, space="PSUM"))

        # resident weights / constants
        wq_sb = wpool.tile([128, KC_D, D], bf16)
        nc.sync.dma_start(out=wq_sb[:], in_=wq_d[:])
        wk_sb = wpool.tile([128, KC_D, D], bf16)
        nc.sync.dma_start(out=wk_sb[:], in_=wk_d[:])
        wv_sb = wpool.tile([128, KC_D, D], bf16)
        nc.sync.dma_start(out=wv_sb[:], in_=wv_d[:])
        wp_sb = wpool.tile([128, KC_D, D], bf16)
        nc.sync.dma_start(out=wp_sb[:], in_=wp_d[:])
        w1_sb = wpool.tile([128, KC_D, DH], bf16)
        nc.sync.dma_start(out=w1_sb[:], in_=w1_d[:])
        w2_sb = wpool.tile([128, KC_H, D], bf16)
        nc.sync.dma_start(out=w2_sb[:], in_=w2_d[:])
        bq_sb = wpool.tile([128, KC_D], f32)
        nc.sync.dma_start(out=bq_sb[:], in_=bq_d[:])
        bk_sb = wpool.tile([128, KC_D], f32)
        nc.sync.dma_start(out=bk_sb[:], in_=bk_d[:])
        b1_sb = wpool.tile([128, KC_H], f32)
        nc.sync.dma_start(out=b1_sb[:], in_=b1_d[:])
        mask_sb = wpool.tile([T, 3 * T], f32)
        nc.sync.dma_start(out=mask_sb[:], in_=mask_d[:])
        id_sb = wpool.tile([128, 128], bf16)
        nc.sync.dma_start(out=id_sb[:], in_=id_d[:])
        eps_sb = wpool.tile([128, 1], f32)
        nc.vector.memset(eps_sb[:], EPS)
        bv_sb = bp_sb = b2_sb = None
        if use_bv:
            bv_sb = wpool.tile([128, D], f32)
            nc.sync.dma_start(out=bv_sb[:], in_=bv_d[:])
        if use_bp:
            bp_sb = wpool.tile([128, D], f32)
            nc.sync.dma_start(out=bp_sb[:], in_=bp_d[:])
        if use_b2:
            b2_sb = wpool.tile([128, D], f32)
            nc.sync.dma_start(out=b2_sb[:], in_=b2_d[:])

        assert D <= nc.vector.BN_STATS_FMAX

        def ln_stats(x_sl, tp, mvall, i):
            st = pst.tile([128, nc.vector.BN_STATS_DIM], f32, tag="bnst")
            nc.vector.bn_stats(out=st[:tp], in_=x_sl)
            nc.vector.bn_aggr(out=mvall[:tp, :, i], in_=st[:tp])

        def ln_finish2(mvall, tp, half, tag):
            # Sqrt + reciprocal for two tiles of the phase
            sl = slice(2 * half, 2 * half + 2)
            rs2 = pst.tile([128, 2], f32, tag=tag + f"rs{half}")
            nc.scalar.activation(out=rs2[:tp], in_=mvall[:tp, 1, sl], func=AF.Sqrt,
                                 bias=eps_sb[:tp], scale=1.0)
            nc.vector.reciprocal(out=rs2[:tp], in_=rs2[:tp])
            nm2 = pst.tile([128, 2], f32, tag=tag + f"nm{half}")
            nc.vector.scalar_tensor_tensor(out=nm2[:tp], in0=mvall[:tp, 0, sl],
                                           scalar=-1.0, in1=rs2[:tp],
                                           op0=OP.mult, op1=OP.mult)
            return rs2, nm2

        def ln_apply(x_sl, tp, rs2, nm2, j, i, dstT, tag):
            xn = pxn.tile([128, D], bf16, tag=tag)
            nc.vector.tensor_scalar(out=xn[:tp], in0=x_sl,
                                    scalar1=rs2[:tp, j:j + 1],
                                    scalar2=nm2[:tp, j:j + 1],
                                    op0=OP.mult, op1=OP.add)
            ps = psq.tile([128, KC_D, 128], bf16, tag="ps_big")
            for c in range(KC_D):
                nc.tensor.transpose(ps[:128, c, :tp],
                                    xn[:tp, ts(c, 128)], id_sb[:tp, :tp])
            nc.vector.tensor_copy(dstT[:, :, i * tp:(i + 1) * tp], ps[:, :, :tp])

        xf = x_d.flatten_outer_dims()
        of = out_d.flatten_outer_dims()

        groups = []
        tok = 0
        nbat = BC
        while nbat > 0:
            nb = min(NB, nbat)
            groups.append((tok, nb))
            tok += nb * T
            nbat -= nb

        live = {}

        def emit_ln1(gi):
            tok0, nb = groups[gi]
            ntok = nb * T
            tp = ntok // 4
            XnT = pxt.tile([128, KC_D, NTOK_MAX], bf16, tag="xnt")
            mvall = pst.tile([128, nc.vector.BN_AGGR_DIM, 4], f32, tag="xn1mv")
            xres = []
            for i in range(4):
                xt_ = px.tile([128, D], f32, tag="xres")
                nc.sync.dma_start(out=xt_[:tp],
                                  in_=xf[tok0 + i * tp: tok0 + (i + 1) * tp])
                xres.append(xt_)
            ln_stats(xres[0][:tp], tp, mvall, 0)
            ln_stats(xres[1][:tp], tp, mvall, 1)
            yield
            ln_stats(xres[2][:tp], tp, mvall, 2)
            ln_stats(xres[3][:tp], tp, mvall, 3)
            # both sqrts back-to-back, early: they land inside the tail's
            # LN2 window while the sqrt ACT-table set is resident
            rsnm = [ln_finish2(mvall, tp, h, "xn1") for h in range(2)]
            yield
            for i in range(4):
                ln_apply(xres[i][:tp], tp, *rsnm[i // 2], i % 2, i, XnT, "xn1")
                yield
            live[gi] = dict(xres=xres, XnT=XnT)
            yield

        def emit_qkv(gi):
            """Q/K/V projections for group gi. Generator (PE-dense filler)."""
            tok0, nb = groups[gi]
            ntok = nb * T
            XnT = live[gi]["XnT"]
            QT = pqk.tile([128, KC_D, NTOK_MAX], bf16, tag="qt")
            KT = pqk.tile([128, KC_D, NTOK_MAX], bf16, tag="kt")
            for di, (dst, w_sb, b_sb) in enumerate(((QT, wq_sb, bq_sb),
                                                    (KT, wk_sb, bk_sb))):
                for m in range(KC_D):
                    ps = psq.tile([128, NTOK_MAX], f32, tag="ps_big")
                    for kc in range(KC_D):
                        nc.tensor.matmul(ps[:, :ntok], w_sb[:, kc, ts(m, 128)],
                                         XnT[:, kc, :ntok],
                                         start=(kc == 0), stop=(kc == KC_D - 1))
                    nc.vector.tensor_scalar_add(out=dst[:, m, :ntok],
                                                in0=ps[:, :ntok],
                                                scalar1=b_sb[:, m:m + 1])
                yield
            V = pv.tile([128, NB, H, VW], bf16, tag="v")
            nc.vector.memset(V[:T, :nb, :, 64:VW], 1.0)
            for b in range(nb):
                ps = pstk.tile([128, D], f32, tag="ps_tok")
                for kc in range(KC_D):
                    nc.tensor.matmul(ps[:T, :], XnT[:, kc, b * T:(b + 1) * T],
                                     wv_sb[:, kc, :],
                                     start=(kc == 0), stop=(kc == KC_D - 1))
                psh = ps[:T].rearrange("p (h s) -> p h s", h=H)
                if use_bv:
                    bvh = bv_sb[:T].rearrange("p (h s) -> p h s", h=H)
                    nc.vector.tensor_add(V[:T, b, :, 0:64], psh, bvh)
                else:
                    nc.vector.tensor_copy(V[:T, b, :, 0:64], psh)
                if b % 2 == 1:
                    yield
            live[gi].update(QT=QT, KT=KT, V=V)
            yield

        def emit_attn(gi):
            """Attention for group gi -> feature-major OT. Yields per batch."""
            tok0, nb = groups[gi]
            QT, KT, V = live[gi]["QT"], live[gi]["KT"], live[gi]["V"]
            OT = pot.tile([128, KC_D, NTOK_MAX], bf16, tag="ot")
            for b in range(nb):
                bs = slice(b * T, (b + 1) * T)
                O_sb = po.tile([128, D], bf16, tag="o")
                rden = pst.tile([128, H], f32, tag="rden")
                for g3 in range(2):
                    # same (h%2) within a triple: one PE row-group, so the
                    # three same-bank matmuls issue sequentially (concurrent
                    # same-bank PSUM writes are a hardware fault)
                    hs3 = [g3, g3 + 2, g3 + 4]
                    ps_s = psa.tile([128, 3, T], f32, tag="ps_att")
                    for j, h in enumerate(hs3):
                        mb, mo = h // 2, (h % 2) * 64
                        nc.tensor.matmul(ps_s[:T, j, :],
                                         KT[mo:mo + 64, mb, bs],
                                         QT[mo:mo + 64, mb, bs],
                                         start=True, stop=True)
                    sm = patt.tile([128, 3 * T], f32, tag="sm")
                    nc.vector.tensor_add(sm[:T], ps_s[:T, :, :], mask_sb[:T])
                    ex = patt.tile([128, 3 * T], bf16, tag="ex")
                    nc.scalar.activation(out=ex[:T], in_=sm[:T], func=AF.Exp)
                    ps_o = psv.tile([128, 3, VW], f32, tag="ps_av")
                    for j, h in enumerate(hs3):
                        nc.tensor.matmul(ps_o[:T, j, :],
                                         ex[:T, ts(j, T)],
                                         V[:T, b, h, :],
                                         start=True, stop=True)
                    nc.vector.reciprocal(out=rden[:T, g3 * 3:(g3 + 1) * 3],
                                         in_=ps_o[:T, :, 64:65])
                    # one mul for the whole triple: rden broadcast along the
                    # 64-wide head slice via a stride-0 inner AP dim
                    rsl = rden[:T, g3 * 3:(g3 + 1) * 3]
                    rb = bass.AP(tensor=rsl.tensor, offset=rsl.offset,
                                 ap=[list(rsl.ap[0]), list(rsl.ap[1]), [0, 64]])
                    osl = O_sb[:T].rearrange("p (c two s) -> p c two s",
                                             two=2, s=64)[:, :, g3, :]
                    nc.vector.tensor_mul(osl, ps_o[:T, :, 0:64], rb)
                ps_t = psa.tile([128, KC_D, T], bf16, tag="ps_att")
                for c in range(KC_D):
                    nc.tensor.transpose(ps_t[:128, c, :],
                                        O_sb[:T, ts(c, 128)], id_sb[:T, :T])
                nc.vector.tensor_copy(OT[:, :, bs], ps_t[:, :, :])
                yield
            live[gi]["OT"] = OT

        def emit_tail(gi):
            """proj+residual, LN2, FFN, store for group gi. Generator."""
            tok0, nb = groups[gi]
            ntok = nb * T
            tp = ntok // 4
            xres, OT = live[gi]["xres"], live[gi]["OT"]
            X2 = px2.tile([128, 4, D], f32, tag="x2")
            mvall = pst.tile([128, nc.vector.BN_AGGR_DIM, 4], f32, tag="xn2mv")
            Xn2T = pxt.tile([128, KC_D, NTOK_MAX], bf16, tag="xn2t")
            rsnm = {}
            for i in range(4):
                ps = pstk.tile([128, D], f32, tag="ps_tok")
                for kc in range(KC_D):
                    nc.tensor.matmul(ps[:tp, :], OT[:, kc, i * tp:(i + 1) * tp],
                                     wp_sb[:, kc, :],
                                     start=(kc == 0), stop=(kc == KC_D - 1))
                if use_bp:
                    nc.vector.tensor_add(ps[:tp, :], ps[:tp, :], bp_sb[:tp, :])
                nc.vector.tensor_add(X2[:tp, i, :], ps[:tp, :], xres[i][:tp, :])
                ln_stats(X2[:tp, i, :], tp, mvall, i)
                if i == 1:
                    rsnm[0] = ln_finish2(mvall, tp, 0, "xn2")
                    ln_apply(X2[:tp, 0, :], tp, *rsnm[0], 0, 0, Xn2T, "xn2")
                if i == 3:
                    ln_apply(X2[:tp, 1, :], tp, *rsnm[0], 1, 1, Xn2T, "xn2")
                    rsnm[1] = ln_finish2(mvall, tp, 1, "xn2")
                    ln_apply(X2[:tp, 2, :], tp, *rsnm[1], 0, 2, Xn2T, "xn2")
                    ln_apply(X2[:tp, 3, :], tp, *rsnm[1], 1, 3, Xn2T, "xn2")
                yield
            hf = 2 * tp
            HT = phid.tile([128, KC_H, NTOK_MAX], bf16, tag="hid")
            for m in range(KC_H):
                ps = psq.tile([128, NTOK_MAX], f32, tag="ps_big")
                for kc in range(KC_D):
                    nc.tensor.matmul(ps[:, :hf], w1_sb[:, kc, ts(m, 128)],
                                     Xn2T[:, kc, :hf],
                                     start=(kc == 0), stop=(kc == KC_D - 1))
                for kc in range(KC_D):
                    nc.tensor.matmul(ps[:, hf:ntok], w1_sb[:, kc, ts(m, 128)],
                                     Xn2T[:, kc, hf:ntok],
                                     start=(kc == 0), stop=(kc == KC_D - 1))
                nc.scalar.activation(out=HT[:, m, :ntok], in_=ps[:, :ntok],
                                     func=AF.Relu, bias=b1_sb[:, m:m + 1],
                                     scale=1.0)
                if m % 2 == 1:
                    yield
            for i in range(4):
                ps = pstk.tile([128, D], f32, tag="ps_tok")
                for kc in range(KC_H):
                    nc.tensor.matmul(ps[:tp, :], HT[:, kc, i * tp:(i + 1) * tp],
                                     w2_sb[:, kc, :],
                                     start=(kc == 0), stop=(kc == KC_H - 1))
                if use_b2:
                    nc.vector.tensor_add(ps[:tp, :], ps[:tp, :], b2_sb[:tp, :])
                ot_ = pout.tile([128, D], f32, tag="outt")
                nc.vector.tensor_add(ot_[:tp, :], ps[:tp, :], X2[:tp, i, :])
                nc.sync.dma_start(out=of[tok0 + i * tp: tok0 + (i + 1) * tp],
                                  in_=ot_[:tp, :])
                yield
            del live[gi]

        def alternate(it_a, it_b):
            while it_a is not None or it_b is not None:
                if it_a is not None:
                    try:
                        next(it_a)
                    except StopIteration:
                        it_a = None
                if it_b is not None:
                    try:
                        next(it_b)
                    except StopIteration:
                        it_b = None

        # software pipeline, per iteration g:
        #   [attention(g-1) x QKV(g)]  then  [tail(g-1) x LN1(g+1)]
        # LN1's PE-light stats run under the FFN-dense tail; attention's
        # gap-prone phase runs under the QKV matmuls.
        for _ in emit_ln1(0):
            pass
        for g in range(len(groups)):
            alternate(emit_attn(g - 1) if g >= 1 else None, emit_qkv(g))
            alternate(emit_tail(g - 1) if g >= 1 else None,
                      emit_ln1(g + 1) if g + 1 < len(groups) else None)
        alternate(emit_attn(len(groups) - 1), None)
        alternate(emit_tail(len(groups) - 1), None)

    nc.compile()
    return nc


def _get_nc(use_bv, use_bp, use_b2):
    key = (use_bv, use_bp, use_b2)
    if key not in _NC_CACHE:
        _NC_CACHE[key] = _build_nc(*key)
    return _NC_CACHE[key]


def _prep_inputs(x, wq, wk, wv, wproj, bproj, w1, b1, w2, b2, g1, beta1, g2, beta2):
    import ml_dtypes
    f = np.float32
    bf = ml_dtypes.bfloat16
    # stack per-head projections into [D, D] with head h at columns h*HS:(h+1)*HS
    wq_f = np.ascontiguousarray(wq.transpose(1, 0, 2).reshape(D, D), dtype=f)
    wk_f = np.ascontiguousarray(wk.transpose(1, 0, 2).reshape(D, D), dtype=f)
    wv_f = np.ascontiguousarray(wv.transpose(1, 0, 2).reshape(D, D), dtype=f)
    scale = np.float32(HS ** 0.5)
    # fold LN1 affine into qkv weights, LN2 affine into w1
    wq_p = (g1[:, None] * wq_f) * scale
    wk_p = g1[:, None] * wk_f
    wv_p = g1[:, None] * wv_f
    w1_p = g2[:, None] * w1
    bq = (beta1 @ wq_f) * scale
    bk = beta1 @ wk_f
    bv = beta1 @ wv_f
    b1_p = b1 + beta2 @ w1
    bp = bproj
    b2_p = b2

    def lay(w, kc):
        # [K, M] -> [128, kc, M] bf16 with K split into kc chunks of 128
        return np.ascontiguousarray(
            np.asarray(w, dtype=f).reshape(kc, 128, w.shape[1]).transpose(1, 0, 2)
        ).astype(bf)

    def layb(bias, kc):
        return np.ascontiguousarray(bias.reshape(kc, 128).T, dtype=f)

    # transposed causal mask, tiled for 3 heads: keep (t >= u)
    maskT = np.full((T, T), MASK_VAL, dtype=f)
    maskT[np.triu_indices(T)] = 0.0
    mask3 = np.ascontiguousarray(np.tile(maskT, (1, 3)))

    shared = {
        "wq_l": lay(wq_p, KC_D), "wk_l": lay(wk_p, KC_D), "wv_l": lay(wv_p, KC_D),
        "wp_l": lay(wproj, KC_D), "w1_l": lay(w1_p, KC_D), "w2_l": lay(w2, KC_H),
        "bq_l": layb(bq, KC_D), "bk_l": layb(bk, KC_D), "b1_l": layb(b1_p, KC_H),
        "mask3": mask3, "ident": np.eye(128, dtype=f).astype(bf),
    }
    use_bv = bool(np.any(bv))
    use_bp = bool(np.any(bp))
    use_b2 = bool(np.any(b2_p))
    if use_bv:
        shared["bv_bc"] = np.ascontiguousarray(np.tile(bv.astype(f), (128, 1)))
    if use_bp:
        shared["bp_bc"] = np.ascontiguousarray(np.tile(np.asarray(bp, f), (128, 1)))
    if use_b2:
        shared["b2_bc"] = np.ascontiguousarray(np.tile(np.asarray(b2_p, f), (128, 1)))
    return shared, (use_bv, use_bp, use_b2)


def kernel(**inputs):
    from concourse.bass_utils import run_bass_kernel_spmd

    x = np.asarray(inputs["x"], dtype=np.float32)
    shared, flags = _prep_inputs(
        x, *[np.asarray(inputs[k], dtype=np.float32) for k in
             ("wq", "wk", "wv", "wproj", "bproj", "w1", "b1", "w2", "b2",
              "g1", "beta1", "g2", "beta2")])
    nc = _get_nc(*flags)
    in_maps = []
    for c in range(N_CORES):
        m = dict(shared)
        m["x"] = np.ascontiguousarray(x[c * BC:(c + 1) * BC])
        in_maps.append(m)
    res = run_bass_kernel_spmd(nc, in_maps, core_ids=list(range(N_CORES)))
    return np.concatenate([res.results[i]["out"] for i in range(N_CORES)], axis=0)


# revision 23
# speedup vs baseline: 1.0887x; 1.0634x over previous
"""Trainium2 Bass kernel for a dense pre-norm transformer block.

Reference semantics (per batch b, seq len T=100, d_model D=384, heads H=6):
  h   = LN(x) * g1 + beta1
  q,k,v = per-head projections of h;  wei = softmax(mask(q k^T * sqrt(64)))
  x2  = x + (wei v) Wp + bp
  out = x2 + relu(LN(x2)*g2+beta2 @ W1 + b1) @ W2 + b2

Distribution: data-parallel over the batch dim across 8 NeuronCores
(64 batches each); weights are replicated. No collectives.

Key design points:
- LN affines are folded into the adjacent matmul weights/biases on the
  host; the sqrt(head_size) score scale is folded into Wq.
- The whole matmul path runs in bf16 (fp32 PSUM accumulation): bf16
  gets full PE rate AND hardware fast-weight-load with overlapped
  LDWEIGHTS (fp32/fp32r matmuls must self-load their stationary
  operand, serializing ~107ns per matmul). Residuals/statistics stay
  fp32.
- Attention avoids per-head transposes: scores are computed transposed
  (K^T-slice stationary x Q^T-slice moving), the softmax denominator
  comes from ones-columns appended to V inside the AV matmul, and the
  1/den scaling happens token-major where it is a per-partition scalar.
  Head triples share a PE row-group so same-bank PSUM writes stay
  sequential (concurrent same-bank writes are a hardware fault).
- Software pipeline: group g's LN1 runs ahead; its Q/K/V projections
  interleave with group g-1's attention so the tensor engine never
  idles long enough for the HAM clock gate to re-throttle. Exp and
  Sqrt live in different ACT table sets, so LN phases are kept
  clustered instead of finely alternating with attention exp calls.
"""

import numpy as np
from contextlib import ExitStack

B, T, D = 512, 100, 384
H, HS = 6, 64
DH = 4 * D                      # FFN hidden 1536
N_CORES = 8
BC = B // N_CORES               # batches per core
EPS = 1e-5
MASK_VAL = -1e9
NB = 5                          # batches per inner group
NTOK_MAX = NB * T               # 500
KC_D = D // 128                 # 3 contraction chunks over D
KC_H = DH // 128                # 12 contraction chunks over DH
VW = 66                         # V columns per head: 64 + 2 ones (denominator)

_NC_CACHE = {}


def _build_nc(use_bv, use_bp, use_b2):
    import concourse.bass as bass
    import concourse.tile as tile
    from concourse import bacc, mybir

    f32 = mybir.dt.float32
    bf16 = mybir.dt.bfloat16
    AF = mybir.ActivationFunctionType
    OP = mybir.AluOpType
    ts = bass.ts

    nc = bacc.Bacc("TRN2", target_bir_lowering=False, debug=False,
                   enable_asserts=True, num_devices=N_CORES)

    x_d = nc.dram_tensor("x", [BC, T, D], f32, kind="ExternalInput").ap()
    wq_d = nc.dram_tensor("wq_l", [128, KC_D, D], bf16, kind="ExternalInput").ap()
    wk_d = nc.dram_tensor("wk_l", [128, KC_D, D], bf16, kind="ExternalInput").ap()
    wv_d = nc.dram_tensor("wv_l", [128, KC_D, D], bf16, kind="ExternalInput").ap()
    wp_d = nc.dram_tensor("wp_l", [128, KC_D, D], bf16, kind="ExternalInput").ap()
    w1_d = nc.dram_tensor("w1_l", [128, KC_D, DH], bf16, kind="ExternalInput").ap()
    w2_d = nc.dram_tensor("w2_l", [128, KC_H, D], bf16, kind="ExternalInput").ap()
    bq_d = nc.dram_tensor("bq_l", [128, KC_D], f32, kind="ExternalInput").ap()
    bk_d = nc.dram_tensor("bk_l", [128, KC_D], f32, kind="ExternalInput").ap()
    b1_d = nc.dram_tensor("b1_l", [128, KC_H], f32, kind="ExternalInput").ap()
    mask_d = nc.dram_tensor("mask3", [T, 3 * T], f32, kind="ExternalInput").ap()
    id_d = nc.dram_tensor("ident", [128, 128], bf16, kind="ExternalInput").ap()
    bv_d = bp_d = b2_d = None
    if use_bv:
        bv_d = nc.dram_tensor("bv_bc", [128, D], f32, kind="ExternalInput").ap()
    if use_bp:
        bp_d = nc.dram_tensor("bp_bc", [128, D], f32, kind="ExternalInput").ap()
    if use_b2:
        b2_d = nc.dram_tensor("b2_bc", [128, D], f32, kind="ExternalInput").ap()
    out_d = nc.dram_tensor("out", [BC, T, D], f32, kind="ExternalOutput").ap()

    with tile.TileContext(nc) as tc, ExitStack() as ctx:
        wpool = ctx.enter_context(tc.tile_pool(name="wpool", bufs=1))
        px = ctx.enter_context(tc.tile_pool(name="px", bufs=12))
        pxn = ctx.enter_context(tc.tile_pool(name="pxn", bufs=2))
        pst = ctx.enter_context(tc.tile_pool(name="pst", bufs=4))
        pxt = ctx.enter_context(tc.tile_pool(name="pxt", bufs=2))
        pqk = ctx.enter_context(tc.tile_pool(name="pqk", bufs=2))
        pv = ctx.enter_context(tc.tile_pool(name="pv", bufs=2))
        patt = ctx.enter_context(tc.tile_pool(name="patt", bufs=4))
        po = ctx.enter_context(tc.tile_pool(name="po", bufs=2))
        pot = ctx.enter_context(tc.tile_pool(name="pot", bufs=1))
        px2 = ctx.enter_context(tc.tile_pool(name="px2", bufs=1))
        phid = ctx.enter_context(tc.tile_pool(name="phid", bufs=1))
        pout = ctx.enter_context(tc.tile_pool(name="pout", bufs=3))
        # PSUM: 8 banks total -> (pool,tag) bufs sum to 8
        psq = ctx.enter_context(tc.tile_pool(name="psq", bufs=2, space="PSUM"))
        pstk = ctx.enter_context(tc.tile_pool(name="pstk", bufs=2, space="PSUM"))
        psa = ctx.enter_context(tc.tile_pool(name="psa", bufs=4, space="PSUM"))
        psv = psa

        # resident weights / constants
        wq_sb = wpool.tile([128, KC_D, D], bf16)
        nc.sync.dma_start(out=wq_sb[:], in_=wq_d[:])
        wk_sb = wpool.tile([128, KC_D, D], bf16)
        nc.sync.dma_start(out=wk_sb[:], in_=wk_d[:])
        wv_sb = wpool.tile([128, KC_D, D], bf16)
        nc.sync.dma_start(out=wv_sb[:], in_=wv_d[:])
        wp_sb = wpool.tile([128, KC_D, D], bf16)
        nc.sync.dma_start(out=wp_sb[:], in_=wp_d[:])
        w1_sb = wpool.tile([128, KC_D, DH], bf16)
        nc.sync.dma_start(out=w1_sb[:], in_=w1_d[:])
        w2_sb = wpool.tile([128, KC_H, D], bf16)
        nc.sync.dma_start(out=w2_sb[:], in_=w2_d[:])
        bq_sb = wpool.tile([128, KC_D], f32)
        nc.sync.dma_start(out=bq_sb[:], in_=bq_d[:])
        bk_sb = wpool.tile([128, KC_D], f32)
        nc.sync.dma_start(out=bk_sb[:], in_=bk_d[:])
        b1_sb = wpool.tile([128, KC_H], f32)
        nc.sync.dma_start(out=b1_sb[:], in_=b1_d[:])
        mask_sb = wpool.tile([T, 3 * T], f32)
        nc.sync.dma_start(out=mask_sb[:], in_=mask_d[:])
        id_sb = wpool.tile([128, 128], bf16)
        nc.sync.dma_start(out=id_sb[:], in_=id_d[:])
        eps_sb = wpool.tile([128, 1], f32)
        nc.vector.memset(eps_sb[:], EPS)
        bv_sb = bp_sb = b2_sb = None
        if use_bv:
            bv_sb = wpool.tile([128, D], f32)
            nc.sync.dma_start(out=bv_sb[:], in_=bv_d[:])
        if use_bp:
            bp_sb = wpool.tile([128, D], f32)
            nc.sync.dma_start(out=bp_sb[:], in_=bp_d[:])
        if use_b2:
            b2_sb = wpool.tile([128, D], f32)
            nc.sync.dma_start(out=b2_sb[:], in_=b2_d[:])

        assert D <= nc.vector.BN_STATS_FMAX

        def ln_stats(x_sl, tp, mvall, i):
            st = pst.tile([128, nc.vector.BN_STATS_DIM], f32, tag="bnst")
            nc.vector.bn_stats(out=st[:tp], in_=x_sl)
            nc.vector.bn_aggr(out=mvall[:tp, :, i], in_=st[:tp])

        def ln_finish2(mvall, tp, half, tag):
            # Sqrt + reciprocal for two tiles of the phase
            sl = slice(2 * half, 2 * half + 2)
            rs2 = pst.tile([128, 2], f32, tag=tag + f"rs{half}")
            nc.scalar.activation(out=rs2[:tp], in_=mvall[:tp, 1, sl], func=AF.Sqrt,
                                 bias=eps_sb[:tp], scale=1.0)
            nc.vector.reciprocal(out=rs2[:tp], in_=rs2[:tp])
            nm2 = pst.tile([128, 2], f32, tag=tag + f"nm{half}")
            nc.vector.scalar_tensor_tensor(out=nm2[:tp], in0=mvall[:tp, 0, sl],
                                           scalar=-1.0, in1=rs2[:tp],
                                           op0=OP.mult, op1=OP.mult)
            return rs2, nm2

        def ln_apply(x_sl, tp, rs2, nm2, j, i, dstT, tag):
            xn = pxn.tile([128, D], bf16, tag=tag)
            nc.vector.tensor_scalar(out=xn[:tp], in0=x_sl,
                                    scalar1=rs2[:tp, j:j + 1],
                                    scalar2=nm2[:tp, j:j + 1],
                                    op0=OP.mult, op1=OP.add)
            ps = psq.tile([128, KC_D, 128], bf16, tag="ps_big")
            for c in range(KC_D):
                nc.tensor.transpose(ps[:128, c, :tp],
                                    xn[:tp, ts(c, 128)], id_sb[:tp, :tp])
            nc.vector.tensor_copy(dstT[:, :, i * tp:(i + 1) * tp], ps[:, :, :tp])

        xf = x_d.flatten_outer_dims()
        of = out_d.flatten_outer_dims()

        groups = []
        tok = 0
        nbat = BC
        while nbat > 0:
            nb = min(NB, nbat)
            groups.append((tok, nb))
            tok += nb * T
            nbat -= nb

        live = {}

        def emit_ln1(gi):
            tok0, nb = groups[gi]
            ntok = nb * T
            tp = ntok // 4
            XnT = pxt.tile([128, KC_D, NTOK_MAX], bf16, tag="xnt")
            mvall = pst.tile([128, nc.vector.BN_AGGR_DIM, 4], f32, tag="xn1mv")
            xres = []
            for i in range(4):
                xt_ = px.tile([128, D], f32, tag="xres")
                nc.sync.dma_start(out=xt_[:tp],
                                  in_=xf[tok0 + i * tp: tok0 + (i + 1) * tp])
                xres.append(xt_)
            ln_stats(xres[0][:tp], tp, mvall, 0)
            ln_stats(xres[1][:tp], tp, mvall, 1)
            yield
            ln_stats(xres[2][:tp], tp, mvall, 2)
            ln_stats(xres[3][:tp], tp, mvall, 3)
            # both sqrts back-to-back, early: they land inside the tail's
            # LN2 window while the sqrt ACT-table set is resident
            rsnm = [ln_finish2(mvall, tp, h, "xn1") for h in range(2)]
            yield
            for i in range(4):
                ln_apply(xres[i][:tp], tp, *rsnm[i // 2], i % 2, i, XnT, "xn1")
                yield
            live[gi] = dict(xres=xres, XnT=XnT)
            yield

        def emit_qkv(gi):
            """Q/K/V projections for group gi. Generator (PE-dense filler)."""
            tok0, nb = groups[gi]
            ntok = nb * T
            XnT = live[gi]["XnT"]
            QT = pqk.tile([128, KC_D, NTOK_MAX], bf16, tag="qt")
            KT = pqk.tile([128, KC_D, NTOK_MAX], bf16, tag="kt")
            for di, (dst, w_sb, b_sb) in enumerate(((QT, wq_sb, bq_sb),
                                                    (KT, wk_sb, bk_sb))):
                for m in range(KC_D):
                    ps = psq.tile([128, NTOK_MAX], f32, tag="ps_big")
                    for kc in range(KC_D):
                        nc.tensor.matmul(ps[:, :ntok], w_sb[:, kc, ts(m, 128)],
                                         XnT[:, kc, :ntok],
                                         start=(kc == 0), stop=(kc == KC_D - 1))
                    nc.vector.tensor_scalar_add(out=dst[:, m, :ntok],
                                                in0=ps[:, :ntok],
                                                scalar1=b_sb[:, m:m + 1])
                yield
            V = pv.tile([128, NB, H, VW], bf16, tag="v")
            nc.vector.memset(V[:T, :nb, :, 64:VW], 1.0)
            for b in range(nb):
                ps = pstk.tile([128, D], f32, tag="ps_tok")
                for kc in range(KC_D):
                    nc.tensor.matmul(ps[:T, :], XnT[:, kc, b * T:(b + 1) * T],
                                     wv_sb[:, kc, :],
                                     start=(kc == 0), stop=(kc == KC_D - 1))
                psh = ps[:T].rearrange("p (h s) -> p h s", h=H)
                if use_bv:
                    bvh = bv_sb[:T].rearrange("p (h s) -> p h s", h=H)
                    nc.vector.tensor_add(V[:T, b, :, 0:64], psh, bvh)
                else:
                    nc.vector.tensor_copy(V[:T, b, :, 0:64], psh)
                if b % 2 == 1:
                    yield
            live[gi].update(QT=QT, KT=KT, V=V)
            yield

        def emit_attn(gi):
            """Attention for group gi -> feature-major OT. Yields per batch."""
            tok0, nb = groups[gi]
            QT, KT, V = live[gi]["QT"], live[gi]["KT"], live[gi]["V"]
            OT = pot.tile([128, KC_D, NTOK_MAX], bf16, tag="ot")

            def front(b):
                bs = slice(b * T, (b + 1) * T)
                exs = []
                for g3 in range(2):
                    # same (h%2) within a triple: one PE row-group, so the
                    # three same-bank matmuls issue sequentially (concurrent
                    # same-bank PSUM writes are a hardware fault)
                    hs3 = [g3, g3 + 2, g3 + 4]
                    ps_s = psa.tile([128, 3, T], f32, tag="ps_att")
                    for j, h in enumerate(hs3):
                        mb, mo = h // 2, (h % 2) * 64
                        nc.tensor.matmul(ps_s[:T, j, :],
                                         KT[mo:mo + 64, mb, bs],
                                         QT[mo:mo + 64, mb, bs],
                                         start=True, stop=True)
                    sm = patt.tile([128, 3 * T], f32, tag="sm")
                    nc.vector.tensor_add(sm[:T], ps_s[:T, :, :], mask_sb[:T])
                    ex = patt.tile([128, 3 * T], bf16, tag="ex")
                    nc.scalar.activation(out=ex[:T], in_=sm[:T], func=AF.Exp)
                    exs.append(ex)
                return exs

            def back(b, exs):
                bs = slice(b * T, (b + 1) * T)
                O_sb = po.tile([128, D], bf16, tag="o")
                rden = pst.tile([128, H], f32, tag="rden")
                for g3 in range(2):
                    hs3 = [g3, g3 + 2, g3 + 4]
                    ex = exs[g3]
                    ps_o = psa.tile([128, 3, VW], f32, tag="ps_att")
                    for j, h in enumerate(hs3):
                        nc.tensor.matmul(ps_o[:T, j, :],
                                         ex[:T, ts(j, T)],
                                         V[:T, b, h, :],
                                         start=True, stop=True)
                    nc.vector.reciprocal(out=rden[:T, g3 * 3:(g3 + 1) * 3],
                                         in_=ps_o[:T, :, 64:65])
                    rsl = rden[:T, g3 * 3:(g3 + 1) * 3]
                    rb = bass.AP(tensor=rsl.tensor, offset=rsl.offset,
                                 ap=[list(rsl.ap[0]), list(rsl.ap[1]), [0, 64]])
                    osl = O_sb[:T].rearrange("p (c two s) -> p c two s",
                                             two=2, s=64)[:, :, g3, :]
                    nc.vector.tensor_mul(osl, ps_o[:T, :, 0:64], rb)
                ps_t = psa.tile([128, KC_D, T], bf16, tag="ps_att")
                for c in range(KC_D):
                    nc.tensor.transpose(ps_t[:128, c, :],
                                        O_sb[:T, ts(c, 128)], id_sb[:T, :T])
                nc.vector.tensor_copy(OT[:, :, bs], ps_t[:, :, :])

            prev = None
            for b in range(nb):
                exs = front(b)
                if prev is not None:
                    back(prev, prev_exs)
                prev, prev_exs = b, exs
                yield
            back(prev, prev_exs)
            live[gi]["OT"] = OT

        def emit_tail(gi):
            """proj+residual, LN2, FFN, store for group gi. Generator."""
            tok0, nb = groups[gi]
            ntok = nb * T
            tp = ntok // 4
            xres, OT = live[gi]["xres"], live[gi]["OT"]
            X2 = px2.tile([128, 4, D], f32, tag="x2")
            mvall = pst.tile([128, nc.vector.BN_AGGR_DIM, 4], f32, tag="xn2mv")
            Xn2T = pxt.tile([128, KC_D, NTOK_MAX], bf16, tag="xn2t")
            rsnm = {}
            for i in range(4):
                ps = pstk.tile([128, D], f32, tag="ps_tok")
                for kc in range(KC_D):
                    nc.tensor.matmul(ps[:tp, :], OT[:, kc, i * tp:(i + 1) * tp],
                                     wp_sb[:, kc, :],
                                     start=(kc == 0), stop=(kc == KC_D - 1))
                if use_bp:
                    nc.vector.tensor_add(ps[:tp, :], ps[:tp, :], bp_sb[:tp, :])
                nc.vector.tensor_add(X2[:tp, i, :], ps[:tp, :], xres[i][:tp, :])
                ln_stats(X2[:tp, i, :], tp, mvall, i)
                if i == 1:
                    rsnm[0] = ln_finish2(mvall, tp, 0, "xn2")
                    ln_apply(X2[:tp, 0, :], tp, *rsnm[0], 0, 0, Xn2T, "xn2")
                if i == 3:
                    ln_apply(X2[:tp, 1, :], tp, *rsnm[0], 1, 1, Xn2T, "xn2")
                    rsnm[1] = ln_finish2(mvall, tp, 1, "xn2")
                    ln_apply(X2[:tp, 2, :], tp, *rsnm[1], 0, 2, Xn2T, "xn2")
                    ln_apply(X2[:tp, 3, :], tp, *rsnm[1], 1, 3, Xn2T, "xn2")
                yield
            hf = 2 * tp
            HT = phid.tile([128, KC_H, NTOK_MAX], bf16, tag="hid")
            for m in range(KC_H):
                ps = psq.tile([128, NTOK_MAX], f32, tag="ps_big")
                for kc in range(KC_D):
                    nc.tensor.matmul(ps[:, :hf], w1_sb[:, kc, ts(m, 128)],
                                     Xn2T[:, kc, :hf],
                                     start=(kc == 0), stop=(kc == KC_D - 1))
                for kc in range(KC_D):
                    nc.tensor.matmul(ps[:, hf:ntok], w1_sb[:, kc, ts(m, 128)],
                                     Xn2T[:, kc, hf:ntok],
                                     start=(kc == 0), stop=(kc == KC_D - 1))
                nc.scalar.activation(out=HT[:, m, :ntok], in_=ps[:, :ntok],
                                     func=AF.Relu, bias=b1_sb[:, m:m + 1],
                                     scale=1.0)
                if m % 2 == 1:
                    yield
            for i in range(4):
                ps = pstk.tile([128, D], f32, tag="ps_tok")
                for kc in range(KC_H):
                    nc.tensor.matmul(ps[:tp, :], HT[:, kc, i * tp:(i + 1) * tp],
                                     w2_sb[:, kc, :],
                                     start=(kc == 0), stop=(kc == KC_H - 1))
                if use_b2:
                    nc.vector.tensor_add(ps[:tp, :], ps[:tp, :], b2_sb[:tp, :])
                ot_ = pout.tile([128, D], f32, tag="outt")
                nc.vector.tensor_add(ot_[:tp, :], ps[:tp, :], X2[:tp, i, :])
                nc.sync.dma_start(out=of[tok0 + i * tp: tok0 + (i + 1) * tp],
                                  in_=ot_[:tp, :])
                yield
            del live[gi]

        def alternate(it_a, it_b):
            while it_a is not None or it_b is not None:
                if it_a is not None:
                    try:
                        next(it_a)
                    except StopIteration:
                        it_a = None
                if it_b is not None:
                    try:
                        next(it_b)
                    except StopIteration:
                        it_b = None

        # software pipeline, per iteration g:
        #   [attention(g-1) x QKV(g)]  then  [tail(g-1) x LN1(g+1)]
        # LN1's PE-light stats run under the FFN-dense tail; attention's
        # gap-prone phase runs under the QKV matmuls.
        for _ in emit_ln1(0):
            pass
        for g in range(len(groups)):
            alternate(emit_attn(g - 1) if g >= 1 else None, emit_qkv(g))
            alternate(emit_tail(g - 1) if g >= 1 else None,
                      emit_ln1(g + 1) if g + 1 < len(groups) else None)
        alternate(emit_attn(len(groups) - 1), None)
        alternate(emit_tail(len(groups) - 1), None)

    nc.compile()
    return nc


def _get_nc(use_bv, use_bp, use_b2):
    key = (use_bv, use_bp, use_b2)
    if key not in _NC_CACHE:
        _NC_CACHE[key] = _build_nc(*key)
    return _NC_CACHE[key]


def _prep_inputs(x, wq, wk, wv, wproj, bproj, w1, b1, w2, b2, g1, beta1, g2, beta2):
    import ml_dtypes
    f = np.float32
    bf = ml_dtypes.bfloat16
    # stack per-head projections into [D, D] with head h at columns h*HS:(h+1)*HS
    wq_f = np.ascontiguousarray(wq.transpose(1, 0, 2).reshape(D, D), dtype=f)
    wk_f = np.ascontiguousarray(wk.transpose(1, 0, 2).reshape(D, D), dtype=f)
    wv_f = np.ascontiguousarray(wv.transpose(1, 0, 2).reshape(D, D), dtype=f)
    scale = np.float32(HS ** 0.5)
    # fold LN1 affine into qkv weights, LN2 affine into w1
    wq_p = (g1[:, None] * wq_f) * scale
    wk_p = g1[:, None] * wk_f
    wv_p = g1[:, None] * wv_f
    w1_p = g2[:, None] * w1
    bq = (beta1 @ wq_f) * scale
    bk = beta1 @ wk_f
    bv = beta1 @ wv_f
    b1_p = b1 + beta2 @ w1
    bp = bproj
    b2_p = b2

    def lay(w, kc):
        # [K, M] -> [128, kc, M] bf16 with K split into kc chunks of 128
        return np.ascontiguousarray(
            np.asarray(w, dtype=f).reshape(kc, 128, w.shape[1]).transpose(1, 0, 2)
        ).astype(bf)

    def layb(bias, kc):
        return np.ascontiguousarray(bias.reshape(kc, 128).T, dtype=f)

    # transposed causal mask, tiled for 3 heads: keep (t >= u)
    maskT = np.full((T, T), MASK_VAL, dtype=f)
    maskT[np.triu_indices(T)] = 0.0
    mask3 = np.ascontiguousarray(np.tile(maskT, (1, 3)))

    shared = {
        "wq_l": lay(wq_p, KC_D), "wk_l": lay(wk_p, KC_D), "wv_l": lay(wv_p, KC_D),
        "wp_l": lay(wproj, KC_D), "w1_l": lay(w1_p, KC_D), "w2_l": lay(w2, KC_H),
        "bq_l": layb(bq, KC_D), "bk_l": layb(bk, KC_D), "b1_l": layb(b1_p, KC_H),
        "mask3": mask3, "ident": np.eye(128, dtype=f).astype(bf),
    }
    use_bv = bool(np.any(bv))
    use_bp = bool(np.any(bp))
    use_b2 = bool(np.any(b2_p))
    if use_bv:
        shared["bv_bc"] = np.ascontiguousarray(np.tile(bv.astype(f), (128, 1)))
    if use_bp:
        shared["bp_bc"] = np.ascontiguousarray(np.tile(np.asarray(bp, f), (128, 1)))
    if use_b2:
        shared["b2_bc"] = np.ascontiguousarray(np.tile(np.asarray(b2_p, f), (128, 1)))
    return shared, (use_bv, use_bp, use_b2)


def kernel(**inputs):
    from concourse.bass_utils import run_bass_kernel_spmd

    x = np.asarray(inputs["x"], dtype=np.float32)
    shared, flags = _prep_inputs(
        x, *[np.asarray(inputs[k], dtype=np.float32) for k in
             ("wq", "wk", "wv", "wproj", "bproj", "w1", "b1", "w2", "b2",
              "g1", "beta1", "g2", "beta2")])
    nc = _get_nc(*flags)
    in_maps = []
    for c in range(N_CORES):
        m = dict(shared)
        m["x"] = np.ascontiguousarray(x[c * BC:(c + 1) * BC])
        in_maps.append(m)
    res = run_bass_kernel_spmd(nc, in_maps, core_ids=list(range(N_CORES)))
    return np.concatenate([res.results[i]["out"] for i in range(N_CORES)], axis=0)
